# revision 7
# baseline (speedup 1.0000x reference)
"""BiATT kernel for 8 Trainium2 NeuronCores.

The reference module's bilinear-attention branch is dead code: the
"attention" weights are softmax(axis=1) over [N, 1] tensors, which is
exactly 1.0 for every row.  Hence

    cf_final = atoms_vector @ (Wcc[0:D] + Wcc[D:2D] + Wcc[2D:3D] + Wcc[3D:4D]) + bcc
    pf_final = amino_vector @ (Wcp[0:D] + Wcp[D:2D] + Wcp[2D:3D] + Wcp[3D:4D]) + bcp

bit-for-bit up to fp32 rounding.

Distribution: 4+4 core split — cores 0-3 compute cf rows (1536 each),
cores 4-7 compute pf rows.  Each core runs one [1536, 512] @ [512, 512]
matmul: 12 row-block PSUM groups x 4 K-chunk matmuls of N=512.  Versus
an 8-way both-streams split this halves the replicated-weight DMA and
keeps one weight tensor per core.

Numerics: single-term fp16 matmul with fp32 PSUM accumulation and fp16
outputs (upcast + rank-1 bias on the host).  Measured end-to-end error
vs the fp32 reference is ~4e-4 (fp16 keeps 11 mantissa bits; the
harness gate is 2e-2).  This is 1/3 the PE work and ~45% of the DMA
bytes of the previous bf16-split-term scheme.  BIATT_DT=bf16 selects
bfloat16 (~3.4e-3) instead.

Schedule (hand-scheduled raw bacc, no Tile framework): input DMAs ride
the Sync HWDGE ring (weight first, then six 256-row x^T pieces in
consumption order), output DMAs the Activation ring.  A burst of
throwaway matmuls on scratch tiles keeps the PE busy during the DMA
lead so the HAM clock gate is released (2.4 GHz) when the real matmul
stream starts.  PSUM bank g%8 holds row-block g; groups 8-11 wait for
the DVE copy of group g-8 before reusing the recycled bank.  The final
group's copy+store is split in two half-width pieces so the second
half's DMA overlaps the first's.
"""

import os
from contextlib import ExitStack

import ml_dtypes
import numpy as np

import concourse.bacc as bacc
import concourse.bass as bass  # noqa: F401  (MemorySpace re-export parity)
import concourse.mybir as mybir
from concourse.bass_utils import run_bass_kernel_spmd

N_CORES = 8
D = 512            # feature dim
N_ROWS = 6144      # rows of atoms_vector / amino_vector
SEG = N_ROWS // 4  # 1536 rows per core (4 cores per stream)
P = 128            # SBUF partitions
KC = D // P        # 4 contraction chunks
RB = SEG // P      # 12 row blocks per core
NPIECE = 6         # x shipped as 6 pieces of 256 rows (2 row blocks)
RPP = SEG // NPIECE  # 256 rows per piece

_F32 = mybir.dt.float32
_PROGRAM_CACHE = {}

_LAST_EXEC_NS = None
_LAST_RES = None


def _new_bass():
    return bacc.Bacc(
        "TRN2",
        target_bir_lowering=False,
        debug=False,
        num_devices=N_CORES,
    )


def _build(dt_name, nwarm):
    dt = mybir.dt.float16 if dt_name == "fp16" else mybir.dt.bfloat16

    nc = _new_bass()

    d_w = nc.dram_tensor("w", [P, KC, D], dt, kind="ExternalInput").ap()
    d_x = [
        nc.dram_tensor(f"x{j}", [P, KC, RPP], dt, kind="ExternalInput").ap()
        for j in range(NPIECE)
    ]
    d_o = nc.dram_tensor("o", [RB, P, D], dt, kind="ExternalOutput").ap()

    with ExitStack() as ctx:
        sb_w = ctx.enter_context(nc.sbuf_tensor("sb_w", [P, KC, D], dt))
        sb_x = [
            ctx.enter_context(nc.sbuf_tensor(f"sb_x{j}", [P, KC, RPP], dt))
            for j in range(NPIECE)
        ]
        outsb = [
            ctx.enter_context(nc.sbuf_tensor(f"outsb{g}", [P, D], dt))
            for g in range(RB)
        ]
        warm = ctx.enter_context(nc.sbuf_tensor("warmsb", [P, 2 * P], dt))
        ps = [
            ctx.enter_context(nc.psum_tensor(f"psum{i}", [P, D], _F32))
            for i in range(8)
        ]
        s_w = ctx.enter_context(nc.semaphore("s_w"))
        s_x = [ctx.enter_context(nc.semaphore(f"s_x{j}")) for j in range(NPIECE)]
        s_mm = ctx.enter_context(nc.semaphore("s_mm"))
        s_cp = ctx.enter_context(nc.semaphore("s_cp"))
        s_wm = ctx.enter_context(nc.semaphore("s_wm"))
        s_ot = ctx.enter_context(nc.semaphore("s_ot"))

        LAST = RB - 1
        H = D // 2

        with nc.Block() as block:

            @block.sync
            def _(sync):
                sync.dma_start(sb_w[:], d_w[:]).then_inc(s_w, 16)
                for j in range(NPIECE):
                    sync.dma_start(sb_x[j][:], d_x[j][:]).then_inc(s_x[j], 16)

            @block.gpsimd
            def _(gpsimd):
                nc.gpsimd.memset(warm[:], 0.0).then_inc(s_wm, 1)

            @block.tensor
            def _(tensor):
                # HAM warm-up on scratch data (bank 7 is reset by group 7's
                # start=True before anything reads it).
                tensor.wait_ge(s_wm, 1)
                for i in range(nwarm):
                    nc.tensor.matmul(
                        ps[7][:, 0:P], warm[:, 0:P], warm[:, P:2 * P],
                        start=(i == 0), stop=(i == nwarm - 1),
                    )
                tensor.wait_ge(s_w, 16)
                for g in range(RB):
                    j, half = divmod(g, 2)
                    if half == 0:
                        tensor.wait_ge(s_x[j], 16)
                    if g >= 8:
                        tensor.wait_ge(s_cp, g - 7)
                    last = None
                    for k in range(KC):
                        last = nc.tensor.matmul(
                            ps[g % 8][:],
                            sb_x[j][:, k, half * P:(half + 1) * P],
                            sb_w[:, k, :],
                            start=(k == 0),
                            stop=(k == KC - 1),
                        )
                    last.then_inc(s_mm, 1)

            @block.vector
            def _(vector):
                for g in range(RB):
                    vector.wait_ge(s_mm, g + 1)
                    if g == LAST:
                        for h in range(2):
                            nc.vector.tensor_copy(
                                outsb[g][:, h * H:(h + 1) * H],
                                ps[g % 8][:, h * H:(h + 1) * H],
                            ).then_inc(s_cp, 1)
                    else:
                        nc.vector.tensor_copy(
                            outsb[g][:], ps[g % 8][:]
                        ).then_inc(s_cp, 1)

            @block.scalar
            def _(scalar):
                for g in range(RB):
                    if g == LAST:
                        for h in range(2):
                            scalar.wait_ge(s_cp, g + 1 + h)
                            scalar.dma_start(
                                d_o[g][:, h * H:(h + 1) * H],
                                outsb[g][:, h * H:(h + 1) * H],
                            ).then_inc(s_ot, 16)
                    else:
                        scalar.wait_ge(s_cp, g + 1)
                        scalar.dma_start(d_o[g], outsb[g][:]).then_inc(s_ot, 16)

        nc.compile()
    return nc


def _get_program(dt_name, nwarm):
    key = (dt_name, nwarm)
    if key not in _PROGRAM_CACHE:
        _PROGRAM_CACHE[key] = _build(dt_name, nwarm)
    return _PROGRAM_CACHE[key]


def _np_dt(dt_name):
    return np.float16 if dt_name == "fp16" else ml_dtypes.bfloat16


def _kchunk(mat_t, np_dt):
    """[K=512, len] -> [128, 4, len] partition-major K-chunked."""
    ln = mat_t.shape[1]
    return np.ascontiguousarray(
        mat_t.astype(np_dt).reshape(KC, P, ln).transpose(1, 0, 2)
    )


def kernel(**inputs):
    global _LAST_EXEC_NS, _LAST_RES

    atoms = np.ascontiguousarray(np.asarray(inputs["atoms_vector"], dtype=np.float32))
    amino = np.ascontiguousarray(np.asarray(inputs["amino_vector"], dtype=np.float32))
    Wcc = np.asarray(inputs["Wcc"], dtype=np.float32)
    Wcp = np.asarray(inputs["Wcp"], dtype=np.float32)
    bcc = np.asarray(inputs["bcc"], dtype=np.float32)
    bcp = np.asarray(inputs["bcp"], dtype=np.float32)

    # Fold the four weight blocks (concat([v]*4, 1) @ W == v @ sum-of-blocks).
    wcc_f = Wcc.reshape(4, D, D).sum(axis=0)
    wcp_f = Wcp.reshape(4, D, D).sum(axis=0)

    dt_name = os.environ.get("BIATT_DT", "fp16")
    nwarm = int(os.environ.get("BIATT_NWARM", "22"))
    np_dt = _np_dt(dt_name)
    nc = _get_program(dt_name, nwarm)

    w_parts = {
        True: _kchunk(wcc_f, np_dt),   # cf stream (cores 0-3)
        False: _kchunk(wcp_f, np_dt),  # pf stream (cores 4-7)
    }
    in_maps = []
    for c in range(N_CORES):
        is_cf = c < 4
        src = atoms if is_cf else amino
        ci = c % 4
        seg_t = _kchunk(src[ci * SEG:(ci + 1) * SEG].T, np_dt)  # [128, 4, 1536]
        m = {"w": w_parts[is_cf]}
        for j in range(NPIECE):
            m[f"x{j}"] = np.ascontiguousarray(seg_t[:, :, j * RPP:(j + 1) * RPP])
        in_maps.append(m)

    trace = bool(os.environ.get("BIATT_TRACE"))
    try:
        res = run_bass_kernel_spmd(nc, in_maps, list(range(N_CORES)), trace=trace)
    except Exception:
        # One retry: a transiently wedged NeuronCore surfaces as a runtime
        # error on an otherwise-valid program.
        res = run_bass_kernel_spmd(nc, in_maps, list(range(N_CORES)), trace=trace)
    _LAST_EXEC_NS = res.exec_time_ns
    _LAST_RES = res

    outs = [
        np.asarray(res.results[c]["o"]).reshape(SEG, D).astype(np.float32)
        for c in range(N_CORES)
    ]
    cf = np.concatenate(outs[:4], axis=0)
    pf = np.concatenate(outs[4:], axis=0)
    cf += bcc  # rank-1 epilogue on the gathered output
    pf += bcp
    return cf, pf


# revision 8
# speedup vs baseline: 1.0158x; 1.0158x over previous
"""BiATT kernel for 8 Trainium2 NeuronCores.

The reference module's bilinear-attention branch is dead code: the
"attention" weights are softmax(axis=1) over [N, 1] tensors, which is
exactly 1.0 for every row.  Hence

    cf_final = atoms_vector @ (Wcc[0:D] + Wcc[D:2D] + Wcc[2D:3D] + Wcc[3D:4D]) + bcc
    pf_final = amino_vector @ (Wcp[0:D] + Wcp[D:2D] + Wcp[2D:3D] + Wcp[3D:4D]) + bcp

bit-for-bit up to fp32 rounding.

Distribution: 4+4 core split — cores 0-3 compute cf rows (1536 each),
cores 4-7 compute pf rows.  Each core runs one [1536, 512] @ [512, 512]
matmul: 12 row-block PSUM groups x 4 K-chunk matmuls of N=512 (back-to-
back warm matmuls measure 216 ns — PE roofline).

Numerics: single-term fp16 matmul with fp32 PSUM accumulation and fp16
outputs (upcast + rank-1 bias on the host).  Measured end-to-end error
vs the fp32 reference is ~5e-4 (the harness gate is 2e-2).
BIATT_DT=bf16 selects bfloat16 (~3.4e-3) instead.

Schedule (hand-scheduled raw bacc, no Tile framework): input DMAs ride
the Activation HWDGE ring (the Scalar engine's NEFF preamble retires
~1 us before Sync's, so first bytes move earlier), output DMAs the Sync
ring.  The weight is shipped as four K-chunk DMAs and the activation as
seven row-pieces (128, 128, then 5 x 256 rows) with per-piece
semaphores, so the first matmul is gated on just 393 KB of traffic and
later pieces stream in behind the PE.  A burst of throwaway matmuls on
a scratch tile keeps the PE busy during the DMA lead so the HAM clock
gate is released (2.4 GHz) near the start of the real matmul stream.
PSUM bank g%8 holds row-block g; groups 8-11 wait for the DVE copy of
group g-8 before reusing the recycled bank.  The final group's
copy+store is split in two half-width pieces so the second half's DMA
overlaps the first's.
"""

import os
from contextlib import ExitStack

import ml_dtypes
import numpy as np

import concourse.bacc as bacc
import concourse.bass as bass  # noqa: F401  (MemorySpace re-export parity)
import concourse.mybir as mybir
from concourse.bass_utils import run_bass_kernel_spmd

N_CORES = 8
D = 512            # feature dim
N_ROWS = 6144      # rows of atoms_vector / amino_vector
SEG = N_ROWS // 4  # 1536 rows per core (4 cores per stream)
P = 128            # SBUF partitions
KC = D // P        # 4 contraction chunks
RB = SEG // P      # 12 row blocks per core

# x row-pieces: two single-row-block pieces first (so the opening gate is
# small), then five 256-row pieces.
PIECE_ROWS = (128, 128, 256, 256, 256, 256, 256)
PIECE_OFF = tuple(sum(PIECE_ROWS[:j]) for j in range(len(PIECE_ROWS)))
NPIECE = len(PIECE_ROWS)
# group -> (piece index, row offset inside the piece)
G_PIECE = []
for _j, _r in enumerate(PIECE_ROWS):
    for _o in range(_r // P):
        G_PIECE.append((_j, _o * P))
assert len(G_PIECE) == RB

_F32 = mybir.dt.float32
_PROGRAM_CACHE = {}

_LAST_EXEC_NS = None
_LAST_RES = None


def _new_bass():
    return bacc.Bacc(
        "TRN2",
        target_bir_lowering=False,
        debug=False,
        num_devices=N_CORES,
    )


def _build(dt_name, nwarm):
    dt = mybir.dt.float16 if dt_name == "fp16" else mybir.dt.bfloat16

    nc = _new_bass()

    d_w = [
        nc.dram_tensor(f"w{k}", [P, D], dt, kind="ExternalInput").ap()
        for k in range(KC)
    ]
    d_x = [
        nc.dram_tensor(f"x{j}", [P, KC, PIECE_ROWS[j]], dt, kind="ExternalInput").ap()
        for j in range(NPIECE)
    ]
    d_o = nc.dram_tensor("o", [RB, P, D], dt, kind="ExternalOutput").ap()

    with ExitStack() as ctx:
        sb_w = [
            ctx.enter_context(nc.sbuf_tensor(f"sb_w{k}", [P, D], dt))
            for k in range(KC)
        ]
        sb_x = [
            ctx.enter_context(
                nc.sbuf_tensor(f"sb_x{j}", [P, KC, PIECE_ROWS[j]], dt)
            )
            for j in range(NPIECE)
        ]
        outsb = [
            ctx.enter_context(nc.sbuf_tensor(f"outsb{g}", [P, D], dt))
            for g in range(RB)
        ]
        warm = ctx.enter_context(nc.sbuf_tensor("warmsb", [P, 2 * P], dt))
        ps = [
            ctx.enter_context(nc.psum_tensor(f"psum{i}", [P, D], _F32))
            for i in range(8)
        ]
        s_w = [ctx.enter_context(nc.semaphore(f"s_w{k}")) for k in range(KC)]
        s_x = [ctx.enter_context(nc.semaphore(f"s_x{j}")) for j in range(NPIECE)]
        s_mm = ctx.enter_context(nc.semaphore("s_mm"))
        s_cp = ctx.enter_context(nc.semaphore("s_cp"))
        s_wm = ctx.enter_context(nc.semaphore("s_wm"))
        s_ot = ctx.enter_context(nc.semaphore("s_ot"))

        LAST = RB - 1
        H = D // 2

        with nc.Block() as block:

            @block.scalar
            def _(scalar):
                # Input DMAs, in consumption order: the first matmul needs
                # only w0 + x0; later weight chunks and row pieces stream
                # in behind the PE.
                scalar.dma_start(sb_w[0][:], d_w[0][:]).then_inc(s_w[0], 16)
                scalar.dma_start(sb_x[0][:], d_x[0][:]).then_inc(s_x[0], 16)
                for k in range(1, KC):
                    scalar.dma_start(sb_w[k][:], d_w[k][:]).then_inc(s_w[k], 16)
                for j in range(1, NPIECE):
                    scalar.dma_start(sb_x[j][:], d_x[j][:]).then_inc(s_x[j], 16)

            @block.gpsimd
            def _(gpsimd):
                nc.gpsimd.memset(warm[:], 0.0).then_inc(s_wm, 1)

            @block.tensor
            def _(tensor):
                # HAM warm-up on scratch data (bank 7 is reset by group 7's
                # start=True before anything reads it).
                tensor.wait_ge(s_wm, 1)
                for i in range(nwarm):
                    nc.tensor.matmul(
                        ps[7][:, 0:P], warm[:, 0:P], warm[:, P:2 * P],
                        start=(i == 0), stop=(i == nwarm - 1),
                    )
                waited = set()

                def gate(sem, key):
                    if key not in waited:
                        waited.add(key)
                        tensor.wait_ge(sem, 16)

                for g in range(RB):
                    j, off = G_PIECE[g]
                    if g >= 8:
                        tensor.wait_ge(s_cp, g - 7)
                    last = None
                    for k in range(KC):
                        gate(s_x[j], ("x", j))
                        gate(s_w[k], ("w", k))
                        last = nc.tensor.matmul(
                            ps[g % 8][:],
                            sb_x[j][:, k, off:off + P],
                            sb_w[k][:],
                            start=(k == 0),
                            stop=(k == KC - 1),
                        )
                    last.then_inc(s_mm, 1)

            @block.vector
            def _(vector):
                for g in range(RB):
                    vector.wait_ge(s_mm, g + 1)
                    if g == LAST:
                        for h in range(2):
                            nc.vector.tensor_copy(
                                outsb[g][:, h * H:(h + 1) * H],
                                ps[g % 8][:, h * H:(h + 1) * H],
                            ).then_inc(s_cp, 1)
                    else:
                        nc.vector.tensor_copy(
                            outsb[g][:], ps[g % 8][:]
                        ).then_inc(s_cp, 1)

            @block.sync
            def _(sync):
                for g in range(RB):
                    if g == LAST:
                        for h in range(2):
                            sync.wait_ge(s_cp, g + 1 + h)
                            sync.dma_start(
                                d_o[g][:, h * H:(h + 1) * H],
                                outsb[g][:, h * H:(h + 1) * H],
                            ).then_inc(s_ot, 16)
                    else:
                        sync.wait_ge(s_cp, g + 1)
                        sync.dma_start(d_o[g], outsb[g][:]).then_inc(s_ot, 16)

        nc.compile()
    return nc


def _get_program(dt_name, nwarm):
    key = (dt_name, nwarm)
    if key not in _PROGRAM_CACHE:
        _PROGRAM_CACHE[key] = _build(dt_name, nwarm)
    return _PROGRAM_CACHE[key]


def _np_dt(dt_name):
    return np.float16 if dt_name == "fp16" else ml_dtypes.bfloat16


def _kchunk(mat_t, np_dt):
    """[K=512, len] -> [128, 4, len] partition-major K-chunked."""
    ln = mat_t.shape[1]
    return np.ascontiguousarray(
        mat_t.astype(np_dt).reshape(KC, P, ln).transpose(1, 0, 2)
    )


def kernel(**inputs):
    global _LAST_EXEC_NS, _LAST_RES

    atoms = np.ascontiguousarray(np.asarray(inputs["atoms_vector"], dtype=np.float32))
    amino = np.ascontiguousarray(np.asarray(inputs["amino_vector"], dtype=np.float32))
    Wcc = np.asarray(inputs["Wcc"], dtype=np.float32)
    Wcp = np.asarray(inputs["Wcp"], dtype=np.float32)
    bcc = np.asarray(inputs["bcc"], dtype=np.float32)
    bcp = np.asarray(inputs["bcp"], dtype=np.float32)

    # Fold the four weight blocks (concat([v]*4, 1) @ W == v @ sum-of-blocks).
    wcc_f = Wcc.reshape(4, D, D).sum(axis=0)
    wcp_f = Wcp.reshape(4, D, D).sum(axis=0)

    dt_name = os.environ.get("BIATT_DT", "fp16")
    nwarm = int(os.environ.get("BIATT_NWARM", "30"))
    np_dt = _np_dt(dt_name)
    nc = _get_program(dt_name, nwarm)

    w_parts = {}
    for is_cf, wf in ((True, wcc_f), (False, wcp_f)):
        wk = wf.astype(np_dt).reshape(KC, P, D)  # [4, 128, 512] K-chunked
        w_parts[is_cf] = {f"w{k}": np.ascontiguousarray(wk[k]) for k in range(KC)}

    in_maps = []
    for c in range(N_CORES):
        is_cf = c < 4
        src = atoms if is_cf else amino
        ci = c % 4
        seg_t = _kchunk(src[ci * SEG:(ci + 1) * SEG].T, np_dt)  # [128, 4, 1536]
        m = dict(w_parts[is_cf])
        for j in range(NPIECE):
            off = PIECE_OFF[j]
            m[f"x{j}"] = np.ascontiguousarray(seg_t[:, :, off:off + PIECE_ROWS[j]])
        in_maps.append(m)

    trace = bool(os.environ.get("BIATT_TRACE"))
    try:
        res = run_bass_kernel_spmd(nc, in_maps, list(range(N_CORES)), trace=trace)
    except Exception:
        # One retry: a transiently wedged NeuronCore surfaces as a runtime
        # error on an otherwise-valid program.
        res = run_bass_kernel_spmd(nc, in_maps, list(range(N_CORES)), trace=trace)
    _LAST_EXEC_NS = res.exec_time_ns
    _LAST_RES = res

    outs = [
        np.asarray(res.results[c]["o"]).reshape(SEG, D).astype(np.float32)
        for c in range(N_CORES)
    ]
    cf = np.concatenate(outs[:4], axis=0)
    pf = np.concatenate(outs[4:], axis=0)
    cf += bcc  # rank-1 epilogue on the gathered output
    pf += bcp
    return cf, pf
